# revision 1
# baseline (speedup 1.0000x reference)
"""GPT2 block on 8 TRN2 NeuronCores.

Sharding: 2 cores per batch element (B=4). Each core computes the full
block for 1024 of its batch's 2048 query tokens (core 2b: queries
[0:1024), core 2b+1: [1024:2048)); K/V are computed for the full
sequence on both cores, so no cross-core communication is needed.

Dataflow: activations feeding matmuls are kept feature-major (x^T:
[feat, tok]) so the PE contracts over features; LayerNorm/residual run
token-major with PE transposes in between. All matmuls use float32r
(~TF32 precision, 4x fp32 throughput); weights are pre-transposed and
pre-rounded to f32r on the host. The 1/sqrt(Dh) score scale is folded
into wq/bq; the V bias is folded into the output-projection bias
(bo2 = bo + wo @ bv, exact because softmax rows sum to 1).
"""
import numpy as np

import concourse.bacc as bacc
import concourse.mybir as mybir
from concourse import tile
from concourse.alu_op_type import AluOpType as alu
from concourse.bass_utils import run_bass_kernel_spmd

F32 = mybir.dt.float32
F32R = mybir.dt.float32r
BF16 = mybir.dt.bfloat16
AF = mybir.ActivationFunctionType
AX = mybir.AxisListType

B, S, D, H, Dh, FF = 4, 2048, 1024, 16, 64, 4096
P = 128
SL = S // 2          # tokens per core (queries)
NKB = S // P         # 16 key blocks
NQC = SL // 512      # 2 query chunks of 512
EPS = 1e-5

_CACHE = {}


def _round_f32r(x):
    x = np.ascontiguousarray(x, dtype=np.float32)
    hi = (x.view(np.uint32) & 0xFFFF0000).view(np.float32)
    lo = (x - hi)
    lo = (lo.view(np.uint32) & 0xFFFF0000).view(np.float32)
    return hi + lo


def _layernorm_tile(nc, sm, xt, xn):
    """Token-major LN core: xn = (xt - mean(xt)) * rsqrt(var + EPS).

    xt: [128, D] fp32 sbuf tile. xn: [128, D] fp32 out tile.
    sm: small-stats pool.
    """
    s1 = sm.tile([P, 1], F32, tag="s1")
    nc.vector.tensor_reduce(s1, xt, axis=AX.X, op=alu.add)
    negmu = sm.tile([P, 1], F32, tag="negmu")
    nc.vector.tensor_scalar(negmu, s1, -1.0 / D, None, op0=alu.mult)
    xc = sm.tile([P, D], F32, tag="xc", bufs=1)
    nc.vector.tensor_scalar(xc, xt, negmu, None, op0=alu.add)
    ssq = sm.tile([P, 1], F32, tag="ssq")
    nc.scalar.activation(xt, xc, AF.Square, accum_out=ssq)
    vv = sm.tile([P, 1], F32, tag="vv")
    nc.vector.tensor_scalar(vv, ssq, 1.0 / D, EPS, op0=alu.mult, op1=alu.add)
    sd = sm.tile([P, 1], F32, tag="sd")
    nc.scalar.sqrt(sd, vv)
    rstd = sm.tile([P, 1], F32, tag="rstd")
    nc.vector.reciprocal(rstd, sd)
    nc.vector.tensor_scalar(xn, xc, rstd, None, op0=alu.mult)


def _build():
    nc = bacc.Bacc(trn_type="TRN2", target_bir_lowering=False, num_devices=8)

    # ---- DRAM I/O ----
    xkv_d = nc.dram_tensor("xkv", [S, D], F32, kind="ExternalInput")
    xq_d = nc.dram_tensor("xq", [SL, D], F32, kind="ExternalInput")
    qpos_d = nc.dram_tensor("qpos", [P, SL], F32, kind="ExternalInput")
    kidx_d = nc.dram_tensor("kidx", [P, NKB], F32, kind="ExternalInput")
    idf_d = nc.dram_tensor("idf", [P, P], F32, kind="ExternalInput")
    idr_d = nc.dram_tensor("idr", [P, P], F32R, kind="ExternalInput")
    ones_d = nc.dram_tensor("ones1", [P, P], F32R, kind="ExternalInput")
    wq_d = nc.dram_tensor("wqt", [D, D], F32R, kind="ExternalInput")
    wk_d = nc.dram_tensor("wkt", [D, D], F32R, kind="ExternalInput")
    wv_d = nc.dram_tensor("wvt", [D, D], F32R, kind="ExternalInput")
    wo_d = nc.dram_tensor("wot", [D, D], F32R, kind="ExternalInput")
    wfc_d = nc.dram_tensor("wfct", [D, FF], F32R, kind="ExternalInput")
    wpj_d = nc.dram_tensor("wpjt", [FF, D], F32R, kind="ExternalInput")
    bq_d = nc.dram_tensor("bqv", [D, 1], F32, kind="ExternalInput")
    bk_d = nc.dram_tensor("bkv", [D, 1], F32, kind="ExternalInput")
    bo_d = nc.dram_tensor("bov", [D, 1], F32, kind="ExternalInput")
    bfc_d = nc.dram_tensor("bfcv", [FF, 1], F32, kind="ExternalInput")
    bpj_d = nc.dram_tensor("bpjv", [D, 1], F32, kind="ExternalInput")
    g1_d = nc.dram_tensor("g1v", [D, 1], F32, kind="ExternalInput")
    b1_d = nc.dram_tensor("b1v", [D, 1], F32, kind="ExternalInput")
    g2_d = nc.dram_tensor("g2v", [D, 1], F32, kind="ExternalInput")
    b2_d = nc.dram_tensor("b2v", [D, 1], F32, kind="ExternalInput")
    out_d = nc.dram_tensor("out", [SL, D], F32, kind="ExternalOutput")
    # DRAM scratch
    xt1_d = nc.dram_tensor("xt1s", [D, S], F32R)
    qts_d = nc.dram_tensor("qts", [D, SL], F32R)
    at_d = nc.dram_tensor("ats", [D, SL], F32R)
    hms_d = nc.dram_tensor("hms", [SL, D], F32)

    ND = D // P   # 8 feature tiles
    NT_KV = S // P   # 16 token tiles
    NT_Q = SL // P   # 8 token tiles

    with tile.TileContext(nc) as tc:
        with tc.tile_pool(name="persist", bufs=1) as pp, \
             tc.tile_pool(name="small", bufs=2) as sm:
            # constants
            idf = pp.tile([P, P], F32, tag="idf")
            nc.sync.dma_start(out=idf, in_=idf_d.ap())
            idr = pp.tile([P, P], F32R, tag="idr")
            nc.sync.dma_start(out=idr, in_=idr_d.ap())

            def load_cols(name, dram, n):
                t = pp.tile([P, n], F32, tag=name)
                for j in range(n):
                    nc.sync.dma_start(out=t[:, j:j + 1],
                                      in_=dram.ap()[j * P:(j + 1) * P, 0:1])
                return t

            bqc = load_cols("bqc", bq_d, ND)
            bkc = load_cols("bkc", bk_d, ND)
            boc = load_cols("boc", bo_d, ND)
            bfcc = load_cols("bfcc", bfc_d, FF // P)
            bpjc = load_cols("bpjc", bpj_d, ND)
            g1c = load_cols("g1c", g1_d, ND)
            b1c = load_cols("b1c", b1_d, ND)
            g2c = load_cols("g2c", g2_d, ND)
            b2c = load_cols("b2c", b2_d, ND)

            # ================= Phase A: LN1(x_kv) -> xt1 (DRAM, feat-major)
            with tc.tile_pool(name="pha", bufs=3) as pa, \
                 tc.tile_pool(name="pha_ps", bufs=4, space="PSUM") as paps:
                for t in range(NT_KV):
                    xt = pa.tile([P, D], F32, tag="xa")
                    nc.sync.dma_start(out=xt, in_=xkv_d.ap()[t * P:(t + 1) * P, :])
                    xn = pa.tile([P, D], F32, tag="xn")
                    _layernorm_tile(nc, sm, xt, xn)
                    for f in range(ND):
                        pt = paps.tile([P, P], F32, tag="pt")
                        nc.tensor.transpose(pt, xn[:, f * P:(f + 1) * P], idf)
                        st = pa.tile([P, P], F32R, tag="st")
                        nc.vector.tensor_scalar(st, pt, g1c[:, f:f + 1],
                                                b1c[:, f:f + 1],
                                                op0=alu.mult, op1=alu.add)
                        nc.sync.dma_start(
                            out=xt1_d.ap()[f * P:(f + 1) * P, t * P:(t + 1) * P],
                            in_=st)

            # ================= Phase B1: K (feat-major) and V (token-major)
            with tc.tile_pool(name="kv", bufs=1) as kvp:
                ones1 = kvp.tile([P, P], F32R, tag="ones1")
                nc.sync.dma_start(out=ones1, in_=ones_d.ap())
                qpos = kvp.tile([P, SL], F32, tag="qpos")
                nc.sync.dma_start(out=qpos, in_=qpos_d.ap())
                kidx = kvp.tile([P, NKB], F32, tag="kidx")
                nc.sync.dma_start(out=kidx, in_=kidx_d.ap())
                kT = [kvp.tile([P, S], F32R, tag=f"kT{m}", name=f"kT{m}") for m in range(ND)]
                # v_sb: per token-tile, 16 heads x (64 v-feats + 1 ones col)
                VW = H * (Dh + 1)  # 1040
                v_sb = kvp.tile([P, NT_KV * VW], F32R, tag="vsb")
                vview = v_sb.rearrange("p (t h e) -> p t h e", t=NT_KV, e=Dh + 1)
                nc.vector.tensor_copy(
                    out=vview[:, :, :, Dh:Dh + 1],
                    in_=ones1[:, 0:1].to_broadcast([P, NT_KV, H, 1]))

                with tc.tile_pool(name="b1w", bufs=1) as wp, \
                     tc.tile_pool(name="b1x", bufs=1) as xp, \
                     tc.tile_pool(name="b1ps", bufs=4, space="PSUM") as psp:
                    for proj in range(2):  # 0: K, 1: V
                        w_d = wk_d if proj == 0 else wv_d
                        wt = []
                        for kc in range(ND):
                            w1 = wp.tile([P, D], F32R, tag=f"w{kc}", name=f"w{proj}_{kc}")
                            nc.sync.dma_start(
                                out=w1, in_=w_d.ap()[kc * P:(kc + 1) * P, :])
                            wt.append(w1)
                        for tch in range(4):  # 512-token chunks
                            xs = []
                            for kc in range(ND):
                                x1 = xp.tile([P, 512], F32R, tag=f"xs{kc}")
                                nc.sync.dma_start(
                                    out=x1,
                                    in_=xt1_d.ap()[kc * P:(kc + 1) * P,
                                                   tch * 512:(tch + 1) * 512])
                                xs.append(x1)
                            if proj == 0:  # K: out [feat 128, tok 512]
                                for m in range(ND):
                                    ps = psp.tile([P, 512], F32, tag="ps")
                                    for kc in range(ND):
                                        nc.tensor.matmul(
                                            ps, wt[kc][:, m * P:(m + 1) * P],
                                            xs[kc], start=(kc == 0),
                                            stop=(kc == ND - 1))
                                    nc.vector.tensor_scalar(
                                        kT[m][:, tch * 512:(tch + 1) * 512],
                                        ps, bkc[:, m:m + 1], None, op0=alu.add)
                            else:  # V: out [tok 128, feat 512]
                                for tt in range(4):
                                    gt = tch * 4 + tt  # global token tile
                                    for nch in range(2):
                                        ps = psp.tile([P, 512], F32, tag="ps")
                                        for kc in range(ND):
                                            nc.tensor.matmul(
                                                ps,
                                                xs[kc][:, tt * P:(tt + 1) * P],
                                                wt[kc][:, nch * 512:(nch + 1) * 512],
                                                start=(kc == 0),
                                                stop=(kc == ND - 1))
                                        pv = ps.rearrange("p (h e) -> p h e", e=Dh)
                                        nc.vector.tensor_copy(
                                            out=vview[:, gt, nch * 8:(nch + 1) * 8, 0:Dh],
                                            in_=pv)

                # ============ Phase A2 + B2: LN1(x_q) -> xqT -> Q -> qts
                with tc.tile_pool(name="xqt", bufs=1) as xqp:
                    xqT = [xqp.tile([P, SL], F32R, tag=f"xqT{f}", name=f"xqT{f}")
                           for f in range(ND)]
                    with tc.tile_pool(name="pha2", bufs=2) as pa2, \
                         tc.tile_pool(name="pha2_ps", bufs=4, space="PSUM") as paps2:
                        for t in range(NT_Q):
                            xt = pa2.tile([P, D], F32, tag="xa2")
                            nc.sync.dma_start(
                                out=xt, in_=xq_d.ap()[t * P:(t + 1) * P, :])
                            xn = pa2.tile([P, D], F32, tag="xn2")
                            _layernorm_tile(nc, sm, xt, xn)
                            for f in range(ND):
                                pt = paps2.tile([P, P], F32, tag="pt2")
                                nc.tensor.transpose(
                                    pt, xn[:, f * P:(f + 1) * P], idf)
                                nc.vector.tensor_scalar(
                                    xqT[f][:, t * P:(t + 1) * P], pt,
                                    g1c[:, f:f + 1], b1c[:, f:f + 1],
                                    op0=alu.mult, op1=alu.add)
                    with tc.tile_pool(name="b2w", bufs=2) as wp2, \
                         tc.tile_pool(name="b2s", bufs=3) as sp2, \
                         tc.tile_pool(name="b2ps", bufs=4, space="PSUM") as psp2:
                        for m in range(ND):
                            wcol = wp2.tile([P, ND * P], F32R, tag="wqcol")
                            for kc in range(ND):
                                nc.sync.dma_start(
                                    out=wcol[:, kc * P:(kc + 1) * P],
                                    in_=wq_d.ap()[kc * P:(kc + 1) * P,
                                                  m * P:(m + 1) * P])
                            for tch in range(2):
                                ps = psp2.tile([P, 512], F32, tag="psq")
                                for kc in range(ND):
                                    nc.tensor.matmul(
                                        ps, wcol[:, kc * P:(kc + 1) * P],
                                        xqT[kc][:, tch * 512:(tch + 1) * 512],
                                        start=(kc == 0), stop=(kc == ND - 1))
                                stq = sp2.tile([P, 512], F32R, tag="stq")
                                nc.vector.tensor_scalar(
                                    stq, ps, bqc[:, m:m + 1], None, op0=alu.add)
                                nc.sync.dma_start(
                                    out=qts_d.ap()[m * P:(m + 1) * P,
                                                   tch * 512:(tch + 1) * 512],
                                    in_=stq)

                # ============ Phase C+D: attention + o-proj, per query chunk
                with tc.tile_pool(name="catt", bufs=1) as cap, \
                     tc.tile_pool(name="cq", bufs=2) as cqp, \
                     tc.tile_pool(name="cprob", bufs=2) as cpp, \
                     tc.tile_pool(name="cps_s", bufs=3, space="PSUM") as pss, \
                     tc.tile_pool(name="cps_o", bufs=2, space="PSUM") as pso, \
                     tc.tile_pool(name="cps_b", bufs=1, space="PSUM") as psb, \
                     tc.tile_pool(name="cps_w", bufs=2, space="PSUM") as psw, \
                     tc.tile_pool(name="cwo", bufs=2) as cwo:
                    for qc in range(NQC):
                        masks = []
                        for kb in range(NKB):
                            mk = cap.tile([P, 512], BF16, tag=f"mk{kb}")
                            nc.vector.tensor_scalar(
                                mk, qpos[:, qc * 512:(qc + 1) * 512],
                                kidx[:, kb:kb + 1], None, op0=alu.is_ge)
                            masks.append(mk)
                        o_nT = [cap.tile([P, 512], F32R, tag=f"onT{f}", name=f"onT{f}")
                                for f in range(ND)]
                        for h in range(H):
                            f, r0 = h // 2, (h % 2) * Dh
                            if r0 == 0:
                                qTf = cqp.tile([P, 512], F32R, tag="qTf")
                                nc.sync.dma_start(
                                    out=qTf,
                                    in_=qts_d.ap()[f * P:(f + 1) * P,
                                                   qc * 512:(qc + 1) * 512])
                            po = pso.tile([Dh + 1, 512], F32, tag="po")
                            for kb in range(NKB):
                                ss = pss.tile([P, 512], F32, tag="ss")
                                nc.tensor.matmul(
                                    ss,
                                    kT[f][r0:r0 + Dh, kb * P:(kb + 1) * P],
                                    qTf[r0:r0 + Dh, :],
                                    start=True, stop=True)
                                ep = cpp.tile([P, 512], F32, tag="ep")
                                nc.scalar.activation(ep, ss, AF.Exp)
                                pr = cpp.tile([P, 512], F32R, tag="pr")
                                nc.vector.tensor_tensor(
                                    out=pr, in0=ep, in1=masks[kb], op=alu.mult)
                                nc.tensor.matmul(
                                    po, v_sb[:, kb * VW + h * (Dh + 1):
                                             kb * VW + (h + 1) * (Dh + 1)],
                                    pr, start=(kb == 0), stop=(kb == NKB - 1))
                            rcp = sm.tile([1, 512], F32R, tag="rcp")
                            with nc.allow_low_precision("f32r recip is ~1e-5"):
                                nc.vector.reciprocal(rcp, po[Dh:Dh + 1, :])
                            pb = psb.tile([Dh, 512], F32, tag="pb")
                            nc.tensor.matmul(pb, ones1[0:1, 0:Dh], rcp,
                                             start=True, stop=True)
                            rb = cpp.tile([Dh, 512], F32, tag="rb")
                            nc.scalar.copy(out=rb, in_=pb)
                            nc.vector.tensor_tensor(
                                out=o_nT[f][r0:r0 + Dh, :],
                                in0=po[0:Dh, :], in1=rb, op=alu.mult)
                        # o-proj for this query chunk -> at scratch
                        for m in range(ND):
                            wcol = cwo.tile([P, ND * P], F32R, tag="wocol")
                            for kc in range(ND):
                                nc.sync.dma_start(
                                    out=wcol[:, kc * P:(kc + 1) * P],
                                    in_=wo_d.ap()[kc * P:(kc + 1) * P,
                                                  m * P:(m + 1) * P])
                            ps = psw.tile([P, 512], F32, tag="psw")
                            for kc in range(ND):
                                nc.tensor.matmul(
                                    ps, wcol[:, kc * P:(kc + 1) * P],
                                    o_nT[kc], start=(kc == 0),
                                    stop=(kc == ND - 1))
                            sta = cpp.tile([P, 512], F32R, tag="sta")
                            nc.vector.tensor_scalar(
                                sta, ps, boc[:, m:m + 1], None, op0=alu.add)
                            nc.sync.dma_start(
                                out=at_d.ap()[m * P:(m + 1) * P,
                                              qc * 512:(qc + 1) * 512],
                                in_=sta)

            # ============ Phases E/F/G
            NF = FF // P  # 32
            with tc.tile_pool(name="gel", bufs=1) as gp:
                gelu = [gp.tile([P, SL], F32R, tag=f"ge{m}", name=f"ge{m}")
                        for m in range(NF)]
                # -------- Phase E: h_mid = xq + attn^T^T ; LN2 -> xl2T
                with tc.tile_pool(name="xl2", bufs=1) as x2p:
                    xl2T = [x2p.tile([P, SL], F32R, tag=f"x2T{f}", name=f"x2T{f}") for f in range(ND)]
                    with tc.tile_pool(name="phe", bufs=3) as pe, \
                         tc.tile_pool(name="phe_ps", bufs=4, space="PSUM") as peps:
                        for t in range(NT_Q):
                            xt = pe.tile([P, D], F32, tag="xe")
                            nc.sync.dma_start(out=xt,
                                              in_=xq_d.ap()[t * P:(t + 1) * P, :])
                            hm = pe.tile([P, D], F32, tag="hm")
                            for f in range(ND):
                                ab = pe.tile([P, P], F32R, tag="ab")
                                nc.sync.dma_start(
                                    out=ab, in_=at_d.ap()[f * P:(f + 1) * P,
                                                          t * P:(t + 1) * P])
                                pt = peps.tile([P, P], F32R, tag="pte_r")
                                nc.tensor.transpose(pt, ab, idr)
                                nc.vector.tensor_tensor(
                                    out=hm[:, f * P:(f + 1) * P],
                                    in0=pt, in1=xt[:, f * P:(f + 1) * P],
                                    op=alu.add)
                            nc.sync.dma_start(
                                out=hms_d.ap()[t * P:(t + 1) * P, :], in_=hm)
                            xn = pe.tile([P, D], F32, tag="xne")
                            _layernorm_tile(nc, sm, hm, xn)
                            for f in range(ND):
                                pt = peps.tile([P, P], F32, tag="pte")
                                nc.tensor.transpose(pt, xn[:, f * P:(f + 1) * P], idf)
                                nc.vector.tensor_scalar(
                                    xl2T[f][:, t * P:(t + 1) * P], pt,
                                    g2c[:, f:f + 1], b2c[:, f:f + 1],
                                    op0=alu.mult, op1=alu.add)

                    # -------- Phase F: gelu = Gelu(wfc @ xl2T + bfc)
                    with tc.tile_pool(name="phf_w", bufs=2) as fwp, \
                         tc.tile_pool(name="phf_ps", bufs=4, space="PSUM") as fps:
                        for m in range(NF):
                            wcol = fwp.tile([P, ND * P], F32R, tag="wfccol")
                            for kc in range(ND):
                                nc.sync.dma_start(
                                    out=wcol[:, kc * P:(kc + 1) * P],
                                    in_=wfc_d.ap()[kc * P:(kc + 1) * P,
                                                   m * P:(m + 1) * P])
                            for tch in range(2):
                                ps = fps.tile([P, 512], F32, tag="psf")
                                for kc in range(ND):
                                    nc.tensor.matmul(
                                        ps, wcol[:, kc * P:(kc + 1) * P],
                                        xl2T[kc][:, tch * 512:(tch + 1) * 512],
                                        start=(kc == 0), stop=(kc == ND - 1))
                                nc.scalar.activation(
                                    gelu[m][:, tch * 512:(tch + 1) * 512],
                                    ps, AF.Gelu_apprx_tanh,
                                    bias=bfcc[:, m:m + 1])

                # ======== Phase G: out = hmid + (wpj @ gelu + bpj)^T
                if True:
                    with tc.tile_pool(name="outb", bufs=1) as obp, \
                         tc.tile_pool(name="phg_w", bufs=4) as gwp, \
                         tc.tile_pool(name="phg_s", bufs=3) as gsp, \
                         tc.tile_pool(name="phg_ps", bufs=2, space="PSUM") as gps, \
                         tc.tile_pool(name="phg_pt", bufs=3, space="PSUM") as gpt:
                        outb = [obp.tile([P, D], F32, tag=f"ob{t}", name=f"ob{t}")
                                for t in range(NT_Q)]
                        for m in range(ND):
                            ps2 = [gps.tile([P, 512], F32, tag="psg0", name="psg0"),
                                   gps.tile([P, 512], F32, tag="psg1", name="psg1")]
                            for kc in range(NF):
                                wb = gwp.tile([P, P], F32R, tag="wpb")
                                nc.sync.dma_start(
                                    out=wb,
                                    in_=wpj_d.ap()[kc * P:(kc + 1) * P,
                                                   m * P:(m + 1) * P])
                                for tch in range(2):
                                    nc.tensor.matmul(
                                        ps2[tch], wb,
                                        gelu[kc][:, tch * 512:(tch + 1) * 512],
                                        start=(kc == 0), stop=(kc == NF - 1))
                            for tch in range(2):
                                ps = ps2[tch]
                                stg = gsp.tile([P, 512], F32R, tag="stg")
                                nc.vector.tensor_scalar(
                                    stg, ps, bpjc[:, m:m + 1], None,
                                    op0=alu.add)
                                for tt in range(4):
                                    t = tch * 4 + tt
                                    pt = gpt.tile([P, P], F32R, tag="ptg")
                                    nc.tensor.transpose(
                                        pt, stg[:, tt * P:(tt + 1) * P], idr)
                                    hb = gsp.tile([P, P], F32, tag="hb")
                                    nc.sync.dma_start(
                                        out=hb,
                                        in_=hms_d.ap()[t * P:(t + 1) * P,
                                                       m * P:(m + 1) * P])
                                    nc.vector.tensor_tensor(
                                        out=outb[t][:, m * P:(m + 1) * P],
                                        in0=pt, in1=hb, op=alu.add)
                        for t in range(NT_Q):
                            nc.sync.dma_start(
                                out=out_d.ap()[t * P:(t + 1) * P, :],
                                in_=outb[t])

    nc.compile()
    return nc


def _prep_inputs(h, ln1_g, ln1_b, wq, bq, wk, bk, wv, bv, wo, bo,
                 ln2_g, ln2_b, wfc, bfc, wproj, bproj):
    f32 = np.float32
    h = np.asarray(h, f32)
    wqt = _round_f32r((np.asarray(wq, f32) / 8.0).T)
    wkt = _round_f32r(np.asarray(wk, f32).T)
    wvt = _round_f32r(np.asarray(wv, f32).T)
    wot = _round_f32r(np.asarray(wo, f32).T)
    wfct = _round_f32r(np.asarray(wfc, f32).T)
    wpjt = _round_f32r(np.asarray(wproj, f32).T)
    bo2 = np.asarray(bo, f32) + np.asarray(wo, f32) @ np.asarray(bv, f32)
    common = {
        "idf": np.eye(P, dtype=f32),
        "idr": np.eye(P, dtype=f32),
        "ones1": np.ones((P, P), f32),
        "kidx": (np.arange(NKB, dtype=f32)[None, :] * P
                 + np.arange(P, dtype=f32)[:, None]).astype(f32),
        "wqt": wqt, "wkt": wkt, "wvt": wvt, "wot": wot,
        "wfct": wfct, "wpjt": wpjt,
        "bqv": (np.asarray(bq, f32) / 8.0).reshape(-1, 1),
        "bkv": np.asarray(bk, f32).reshape(-1, 1),
        "bov": bo2.reshape(-1, 1),
        "bfcv": np.asarray(bfc, f32).reshape(-1, 1),
        "bpjv": np.asarray(bproj, f32).reshape(-1, 1),
        "g1v": np.asarray(ln1_g, f32).reshape(-1, 1),
        "b1v": np.asarray(ln1_b, f32).reshape(-1, 1),
        "g2v": np.asarray(ln2_g, f32).reshape(-1, 1),
        "b2v": np.asarray(ln2_b, f32).reshape(-1, 1),
    }
    in_maps = []
    for c in range(8):
        b, j = c // 2, c % 2
        qp = (j * SL + np.arange(SL, dtype=f32))[None, :].repeat(P, axis=0)
        m = dict(common)
        m["xkv"] = np.ascontiguousarray(h[b])
        m["xq"] = np.ascontiguousarray(h[b, j * SL:(j + 1) * SL])
        m["qpos"] = np.ascontiguousarray(qp)
        in_maps.append(m)
    return in_maps


def _run(in_maps, trace=False):
    if "nc" not in _CACHE:
        _CACHE["nc"] = _build()
    return run_bass_kernel_spmd(_CACHE["nc"], in_maps,
                                core_ids=list(range(8)), trace=trace)


def kernel(**inputs):
    in_maps = _prep_inputs(**inputs)
    res = _run(in_maps)
    out = np.empty((B, S, D), np.float32)
    for c in range(8):
        b, j = c // 2, c % 2
        out[b, j * SL:(j + 1) * SL] = res.results[c]["out"]
    return out

